# revision 5
# baseline (speedup 1.0000x reference)
"""MoE top-2 routed expert MLP on 8 Trainium2 NeuronCores.

Strategy (expert-parallel, host-routed):
  - Host computes the (tiny) gate in float64: logits = x @ Wg + bg, top-2,
    softmax combine weights. fp64 ordering reproduces jax's fp32 top_k
    selection exactly for this problem's data (verified).
  - Tokens are dispatched by expert id: core e receives exactly the tokens
    routed to expert e (padded to a common capacity C), plus ONLY expert e's
    W0/W1/W2 — the "shard W* along the expert axis, all-to-all dispatch
    tokens" plan, with the dispatch done host-side.
  - Each core runs a dense [C,1024] -> silu-gated MLP -> [C,1024] in fp32r
    (full PE rate at moving-dim >= 256, ~1.5e-4 matmul rel-err).
  - Host applies combine weights and scatter-adds the two expert outputs
    per token. Expert biases b0/b1 are folded into the on-device epilogues
    only when nonzero; b2's contribution (combine-weighted) is added on host.

Device kernel layout (per core, everything fp32/fp32r):
  xt  [128, 8, C]      x gathered+transposed: xt[p, k, t] = x[tok_t, 128k+p]
  w0  [32, 128, 8, 128] w0[hh, p, k, c]  = W0[e][128k+p, 128hh+c]
  w1  same as w0
  w2  [8, 128, 32, 128] w2[dd, p, hh, c] = W2[e][128hh+p, 128dd+c]
  y   [128, 8, C]      y[p, dd, t] = out[tok_t, 128dd+p]
  mm1: hT[hh-chunk] = W0-block.T @ xt-block accumulated over k (8), PSUM [128, tn]
  hg  = (hT (+b0)) * silu(gT (+b1)) stored as [128, 8, C] per hh-group (4 groups)
  mm2: yT[dd-chunk] += W2-block.T @ hg-block accumulated over the group's 8
       hh-chunks, accumulated across groups in SBUF.
"""
import numpy as np
from contextlib import ExitStack

import concourse.bacc as bacc
import concourse.tile as tile
from concourse import mybir
from concourse.bass_utils import run_bass_kernel_spmd

P = 128
D_MODEL = 1024
D_HID = 4096
E = 8
TOP_K = 2
KC = D_MODEL // P   # 8  contraction chunks for mm1
HH = D_HID // P     # 32 hidden chunks
DD = D_MODEL // P   # 8  output chunks
GHH = 8             # hh-chunks per group
NG = HH // GHH      # 4 groups
F32R = mybir.dt.float32r
F32 = mybir.dt.float32

_BUILD_CACHE = {}
_LAST = {}  # stash of the last BassKernelResults (for external harnesses)


def _token_tiles(C):
    """Split C (even) into even tiles each in [256, 512].

    Even sizes are an fp32r matmul ISA requirement (innermost n_step even);
    >=256 keeps fp32r at full PE rate, <=512 fits one PSUM bank.
    """
    assert C % 2 == 0
    n = -(-C // 512)
    while True:
        base, rem = divmod(C // 2, n)
        sizes = [2 * (base + 1)] * rem + [2 * base] * (n - rem)
        if all(256 <= s <= 512 for s in sizes):
            return sizes
        n += 1


def _build(C, has_b0, has_b1):
    key = (C, has_b0, has_b1)
    if key in _BUILD_CACHE:
        return _BUILD_CACHE[key]

    tiles = _token_tiles(C)
    offs = np.concatenate([[0], np.cumsum(tiles)]).tolist()

    nc = bacc.Bacc()
    xt = nc.declare_dram_parameter("xt", (P, KC, C), F32R, isOutput=False)
    w0 = nc.declare_dram_parameter("w0", (HH, P, KC, P), F32R, isOutput=False)
    w1 = nc.declare_dram_parameter("w1", (HH, P, KC, P), F32R, isOutput=False)
    w2 = nc.declare_dram_parameter("w2", (DD, P, HH, P), F32R, isOutput=False)
    if has_b0:
        b0 = nc.declare_dram_parameter("b0", (P, HH), F32, isOutput=False)
    if has_b1:
        b1 = nc.declare_dram_parameter("b1", (P, HH), F32, isOutput=False)
    y = nc.declare_dram_parameter("y", (P, DD, C), F32, isOutput=True)

    with ExitStack() as ctx:
        tc = ctx.enter_context(tile.TileContext(nc))
        xpool = ctx.enter_context(tc.tile_pool(name="x", bufs=1))
        ypool = ctx.enter_context(tc.tile_pool(name="y", bufs=1))
        hgpool = ctx.enter_context(tc.tile_pool(name="hg", bufs=2))
        wpool = ctx.enter_context(tc.tile_pool(name="w", bufs=3))
        tpool = ctx.enter_context(tc.tile_pool(name="t", bufs=4))
        psh = ctx.enter_context(tc.tile_pool(name="psh", bufs=3, space="PSUM"))
        psg = ctx.enter_context(tc.tile_pool(name="psg", bufs=3, space="PSUM"))
        psy = ctx.enter_context(tc.tile_pool(name="psy", bufs=2, space="PSUM"))

        xts = xpool.tile([P, KC, C], F32R, tag="xt")
        nc.sync.dma_start(xts[:], xt[:])
        ysb = ypool.tile([P, DD, C], F32, tag="ysb")
        if has_b0:
            b0t = xpool.tile([P, HH], F32, tag="b0")
            nc.sync.dma_start(b0t[:], b0[:])
        if has_b1:
            b1t = xpool.tile([P, HH], F32, tag="b1")
            nc.sync.dma_start(b1t[:], b1[:])

        for g in range(NG):
            hgt = hgpool.tile([P, GHH, C], F32R, tag="hgt")
            for hj in range(GHH):
                hh = g * GHH + hj
                w0t = wpool.tile([P, KC, P], F32R, tag="w0")
                nc.sync.dma_start(w0t[:], w0[hh])
                w1t = wpool.tile([P, KC, P], F32R, tag="w1")
                nc.sync.dma_start(w1t[:], w1[hh])
                for it, tn in enumerate(tiles):
                    t0 = offs[it]
                    ps_h = psh.tile([P, tn], F32, tag="ph")
                    for k in range(KC):
                        nc.tensor.matmul(
                            ps_h[:], w0t[:, k], xts[:, k, t0:t0 + tn],
                            start=(k == 0), stop=(k == KC - 1),
                        )
                    ps_g = psg.tile([P, tn], F32, tag="pg")
                    for k in range(KC):
                        nc.tensor.matmul(
                            ps_g[:], w1t[:, k], xts[:, k, t0:t0 + tn],
                            start=(k == 0), stop=(k == KC - 1),
                        )
                    gact = tpool.tile([P, tn], F32, tag="gact")
                    nc.scalar.activation(
                        gact[:], ps_g[:], mybir.ActivationFunctionType.Silu,
                        bias=b1t[:, hh:hh + 1] if has_b1 else 0.0,
                    )
                    h_src = ps_h
                    if has_b0:
                        h_tmp = tpool.tile([P, tn], F32, tag="htmp")
                        nc.vector.tensor_tensor(
                            h_tmp[:], ps_h[:],
                            b0t[:, hh:hh + 1].to_broadcast((P, tn)),
                            mybir.AluOpType.add,
                        )
                        h_src = h_tmp
                    nc.vector.tensor_tensor(
                        hgt[:, hj, t0:t0 + tn], h_src[:], gact[:],
                        mybir.AluOpType.mult,
                    )
            for dd in range(DD):
                w2t = wpool.tile([P, GHH, P], F32R, tag="w2")
                nc.sync.dma_start(w2t[:], w2[dd][:, g * GHH:(g + 1) * GHH])
                for it, tn in enumerate(tiles):
                    t0 = offs[it]
                    ps_y = psy.tile([P, tn], F32, tag="py")
                    for hj in range(GHH):
                        nc.tensor.matmul(
                            ps_y[:], w2t[:, hj], hgt[:, hj, t0:t0 + tn],
                            start=(hj == 0), stop=(hj == GHH - 1),
                        )
                    if g == 0:
                        nc.scalar.copy(ysb[:, dd, t0:t0 + tn], ps_y[:])
                    else:
                        nc.vector.tensor_add(
                            ysb[:, dd, t0:t0 + tn], ysb[:, dd, t0:t0 + tn], ps_y[:]
                        )
                if g == NG - 1:
                    nc.sync.dma_start(y[:, dd], ysb[:, dd])
    nc.finalize()
    _BUILD_CACHE[key] = nc
    return nc


def kernel(x, Wg, bg, W0, b0, W1, b1, W2, b2):
    x = np.asarray(x, dtype=np.float32)
    Wg = np.asarray(Wg, dtype=np.float32)
    bg = np.asarray(bg, dtype=np.float32)
    W0 = np.asarray(W0, dtype=np.float32)
    b0 = np.asarray(b0, dtype=np.float32)
    W1 = np.asarray(W1, dtype=np.float32)
    b1 = np.asarray(b1, dtype=np.float32)
    W2 = np.asarray(W2, dtype=np.float32)
    b2 = np.asarray(b2, dtype=np.float32)

    n, s, d = x.shape
    T = n * s
    xf = x.reshape(T, d)

    # ---- host routing (float64; tie order matches jax.lax.top_k) ----
    gl = xf.astype(np.float64) @ Wg.astype(np.float64) + bg.astype(np.float64)
    ti = np.argsort(-gl, axis=1, kind="stable")[:, :TOP_K]          # [T, K]
    tv = np.take_along_axis(gl, ti, axis=1)
    w = np.exp(tv - tv.max(axis=1, keepdims=True))
    w /= w.sum(axis=1, keepdims=True)                               # [T, K]

    eflat = ti.ravel()
    tflat = np.repeat(np.arange(T), TOP_K)
    wflat = w.ravel()
    order = np.argsort(eflat, kind="stable")
    counts = np.bincount(eflat, minlength=E)
    starts = np.concatenate([[0], np.cumsum(counts)])

    C = max(int(counts.max()), 256)
    C = (C + 7) // 8 * 8
    nc = _build(C, bool(np.any(b0)), bool(np.any(b1)))

    in_maps = []
    core_toks = []
    core_ws = []
    for e in range(E):
        sel = order[starts[e]:starts[e + 1]]
        toks = tflat[sel]
        core_toks.append(toks)
        core_ws.append(wflat[sel])
        toks_pad = np.concatenate([toks, np.zeros(C - len(toks), dtype=np.int64)])
        Xg = xf[toks_pad]                                           # [C, D]
        xtb = np.ascontiguousarray(Xg.T.reshape(KC, P, C).transpose(1, 0, 2))
        w0b = np.ascontiguousarray(W0[e].reshape(KC, P, HH, P).transpose(2, 1, 0, 3))
        w1b = np.ascontiguousarray(W1[e].reshape(KC, P, HH, P).transpose(2, 1, 0, 3))
        w2b = np.ascontiguousarray(W2[e].reshape(HH, P, DD, P).transpose(2, 1, 0, 3))
        m = {"xt": xtb, "w0": w0b, "w1": w1b, "w2": w2b}
        if np.any(b0):
            m["b0"] = np.ascontiguousarray(b0[e].reshape(HH, P).T)
        if np.any(b1):
            m["b1"] = np.ascontiguousarray(b1[e].reshape(HH, P).T)
        in_maps.append(m)

    res = run_bass_kernel_spmd(nc, in_maps, list(range(E)))
    _LAST["res"] = res

    # ---- host combine ----
    out_flat = np.zeros((T, d), dtype=np.float64)
    for e in range(E):
        cnt = counts[e]
        if cnt == 0:
            continue
        ye = res.results[e]["y"].reshape(P, DD, C)                  # [p, dd, t]
        ye = ye.transpose(2, 1, 0).reshape(C, d)[:cnt]              # [cnt, D]
        out_flat[core_toks[e]] += core_ws[e][:, None] * ye
    if np.any(b2):
        out_flat += (w[:, :, None] * b2[ti]).sum(axis=1)

    return out_flat.reshape(n, s, d).astype(np.float32)
